# revision 17
# baseline (speedup 1.0000x reference)
"""Trainium2 Bass kernel for the bilevel logit-assignment flow problem.

Reference math (N=384, cutoff-2 paths):
    A = (adj > 0) & ~eye
    E = A * exp(-lam * dist)                       # "edge weight" matrix
    Z = E + offdiag(E @ E)                          # softmax denominator (m cancels)
    W = where(Z > 0, demand / Z, 0),  demand = relu(od) (diag auto-zero via Z)
    flows = W*E + E*(W @ E^T) + E*(E^T @ W)

Sharding: origin axis o split across 8 cores (48 rows each). Each core
holds full E / E^T (N x N is small), computes its row-slice of Z/W and
the three matmuls, and returns:
    rows [48,384] = E_s * (W_s + W_s @ E^T)        # terms 1+2, rows S
    p3   [384,384] = E * (E_s^T @ W_s)             # term 3 partial (sum over its o-slice)
Host gather: flows = sum_i p3_i; flows[S_i] += rows_i.

Input marshaling (host side, layout only): adjacency is repacked to
uint8 (binary matrix, 4x fewer DMA bytes) and adj/dist/p3 use a
partition-tiled [128, 3, 384] DRAM layout so each partition's DMA row
is contiguous.
"""

import numpy as np

import concourse.bass as bass
import concourse.mybir as mybir
import concourse.tile as tile
from concourse import bacc
from concourse.bass_utils import run_bass_kernel_spmd
from concourse.masks import make_identity

N = 384
NCORES = 8
S = N // NCORES  # 48 origins per core
P = 128
NT = N // P  # 3 partition tiles

F32 = mybir.dt.float32
F32R = mybir.dt.float32r
I32 = mybir.dt.int32
U8 = mybir.dt.uint8
Act = mybir.ActivationFunctionType
Alu = mybir.AluOpType

USE_F32R = True  # fp32r: 1 cyc/row matmul (vs 4 for fp32), operands f32r-rounded


def build_program(lam: float, use_f32r: bool = USE_F32R) -> bass.Bass:
    nc = bacc.Bacc(
        "TRN2",
        target_bir_lowering=False,
        debug=False,
        num_devices=NCORES,
        enable_asserts=False,
    )

    def asmm(ap):
        """View an SBUF AP in the dtype fed to the tensor engine."""
        return ap.bitcast(F32R) if use_f32r else ap

    # partition-tiled layouts: [p, t, n] == full[128*t + p, n]
    adj = nc.dram_tensor("adj_u8t", [P, NT, N], U8, kind="ExternalInput")
    dist = nc.dram_tensor("dist_t", [P, NT, N], F32, kind="ExternalInput")
    # per-core slice pack: [adj_s bits, dist_s, od_s, noteye_s] as f32 planes
    aux_s = nc.dram_tensor("aux_s", [4, S, N], F32, kind="ExternalInput")
    p3 = nc.dram_tensor("p3_t", [P, NT, N], F32, kind="ExternalOutput")
    rows = nc.dram_tensor("rows", [S, N], F32, kind="ExternalOutput")

    aux_r = aux_s.rearrange("k s n -> s k n")  # [48, 4, 384]

    with tile.TileContext(nc) as tc:
        with (
            tc.tile_pool(name="persist", bufs=1) as sb,
            tc.tile_pool(name="work", bufs=2) as work,
            tc.tile_pool(name="pst", bufs=2, space="PSUM") as pst,
            tc.tile_pool(name="psacc", bufs=1, space="PSUM") as psacc,
        ):
            ident = sb.tile([P, P], F32)
            make_identity(nc, ident[:])
            ident_mm = sb.tile([P, P], F32)
            nc.vector.tensor_copy(asmm(ident_mm[:]), ident[:])

            # ---- loads (issue split across the two HWDGE engines) ----
            adj_t = work.tile([P, NT, N], U8, tag="adj_t")
            dist_sb = work.tile([P, NT, N], F32, tag="dist_sb")
            aux = sb.tile([S, 4, N], F32)
            nc.sync.dma_start(adj_t[:], adj[:])
            nc.scalar.dma_start(aux[:], aux_r)
            nc.sync.dma_start(dist_sb[:, 0, :], dist[:, 0, :])
            nc.scalar.dma_start(dist_sb[:, 1, :], dist[:, 1, :])
            nc.sync.dma_start(dist_sb[:, 2, :], dist[:, 2, :])

            # ---- per-core row slice E_s (first: only needs aux) ----
            ne_t = aux[:, 3, :]
            Es = sb.tile([S, N], F32)
            adjsf = work.tile([S, N], F32, tag="adjsf")
            nc.vector.tensor_copy(adjsf[:], aux[:, 0, :].bitcast(I32))
            nc.vector.tensor_mul(adjsf[:], adjsf[:], ne_t)  # zero diag (core-dep)
            expds = work.tile([S, N], F32, tag="expds")
            nc.scalar.activation(expds[:], aux[:, 1, :], Act.Exp, scale=-lam)
            nc.vector.tensor_mul(asmm(Es[:]), adjsf[:], expds[:])

            # E_s^T [N, S] as NT chunks of [128, S]
            EsT = sb.tile([P, NT, S], F32)
            for c in range(NT):
                tp2 = pst.tile([P, S], F32, tag="tp2")
                nc.tensor.transpose(
                    asmm(tp2[:]),
                    asmm(Es[:, P * c : P * (c + 1)]),
                    asmm(ident_mm[:S, :S]),
                )
                nc.vector.tensor_copy(asmm(EsT[:, c, :]), tp2[:])

            # ---- full E (pipelined per row-tile), mm(i) folded in ----
            E = sb.tile([P, NT, N], F32)   # E[p, t, :] == E_full[128*t + p, :]
            ET = sb.tile([P, NT, N], F32)  # ET[p, u, :] == E_full[:, 128*u + p].T
            adjf = work.tile([P, NT, N], F32, tag="adjf")
            expd = work.tile([P, NT, N], F32, tag="expd")
            EEs = psacc.tile([S, N], F32, tag="EEs")
            for t in range(NT):
                nc.vector.tensor_copy(adjf[:, t, :], adj_t[:, t, :])  # u8 -> f32
                # zero the global diagonal: iota = 128*t + p - y
                nc.gpsimd.affine_select(
                    out=adjf[:, t, :],
                    in_=adjf[:, t, :],
                    compare_op=Alu.not_equal,
                    fill=0.0,
                    base=P * t,
                    channel_multiplier=1,
                    pattern=[[-1, N]],
                )
                nc.scalar.activation(expd[:, t, :], dist_sb[:, t, :], Act.Exp, scale=-lam)
                nc.vector.tensor_mul(asmm(E[:, t, :]), adjf[:, t, :], expd[:, t, :])
                # (i) EEs = (E @ E)[S, :], accumulated as E row-tiles complete
                nc.tensor.matmul(
                    EEs[:],
                    asmm(EsT[:, t, :]),
                    asmm(E[:, t, :]),
                    start=(t == 0),
                    stop=(t == NT - 1),
                )

            # ET transposes, u-major; pairs share a PSUM tile to halve the copies
            for u in range(NT):
                tp = pst.tile([P, 2, P], F32, tag="tp")
                for t in range(2):
                    nc.tensor.transpose(
                        asmm(tp[:, t, :]),
                        asmm(E[:, t, P * u : P * (u + 1)]),
                        asmm(ident_mm[:]),
                    )
                nc.vector.tensor_copy(asmm(ET[:, u, 0 : 2 * P]), tp[:])
                tp1 = pst.tile([P, 2, P], F32, tag="tp")
                nc.tensor.transpose(
                    asmm(tp1[:, 0, :]),
                    asmm(E[:, 2, P * u : P * (u + 1)]),
                    asmm(ident_mm[:]),
                )
                nc.vector.tensor_copy(asmm(ET[:, u, 2 * P : N]), tp1[:, 0, :])

            # ---- Z, W (whole-tensor chain; 1-input pieces on gpsimd) ----
            dem = work.tile([S, N], F32, tag="dem")
            nc.vector.tensor_relu(dem[:], aux[:, 2, :])
            Zs = sb.tile([S, N], F32)
            mask = work.tile([S, N], F32, tag="mask")
            zinv = work.tile([S, N], F32, tag="zinv")
            W = sb.tile([S, N], F32)
            WsT = sb.tile([P, NT, S], F32)
            T2 = psacc.tile([S, N], F32, tag="T2")
            nc.vector.tensor_add(Zs[:], Es[:], EEs[:])
            nc.vector.tensor_mul(Zs[:], Zs[:], ne_t)  # offdiag()
            nc.vector.tensor_single_scalar(mask[:], Zs[:], 0.0, Alu.is_gt)
            nc.vector.tensor_mul(dem[:], dem[:], mask[:])
            nc.vector.tensor_scalar_max(Zs[:], Zs[:], 1e-30)
            nc.vector.reciprocal(zinv[:], Zs[:])
            nc.vector.tensor_mul(asmm(W[:]), dem[:], zinv[:])
            for c in range(NT):
                cs = slice(P * c, P * (c + 1))
                tp2 = pst.tile([P, S], F32, tag="tp2")
                nc.tensor.transpose(
                    asmm(tp2[:]), asmm(W[:, cs]), asmm(ident_mm[:S, :S])
                )
                nc.vector.tensor_copy(asmm(WsT[:, c, :]), tp2[:])
                # (ii) T2 = W_s @ E^T
                nc.tensor.matmul(
                    T2[:],
                    asmm(WsT[:, c, :]),
                    asmm(ET[:, c, :]),
                    start=(c == 0),
                    stop=(c == NT - 1),
                )

            # ---- (iii) P3 = E_s^T @ W_s, p3 = E * P3 (early, per-tile out) ----
            for mt in range(NT):
                P3 = pst.tile([P, N], F32, tag="P3")
                nc.tensor.matmul(
                    P3[:],
                    asmm(Es[:, P * mt : P * (mt + 1)]),
                    asmm(W[:]),
                    start=True,
                    stop=True,
                )
                out_t = work.tile([P, N], F32, tag="out_t")
                nc.vector.tensor_mul(out_t[:], E[:, mt, :], P3[:])
                eng = nc.sync if mt % 2 == 0 else nc.scalar
                eng.dma_start(p3[:, mt, :], out_t[:])

            # ---- rows out ----
            rows_sb = work.tile([S, N], F32, tag="rows_sb")
            nc.vector.tensor_add(rows_sb[:], W[:], T2[:])
            nc.vector.tensor_mul(rows_sb[:], rows_sb[:], Es[:])
            nc.scalar.dma_start(rows[:, :], rows_sb[:])

    nc.compile()  # bacc register allocation / DCE / lowering
    return nc


_PROGRAM_CACHE: dict = {}


def _get_program(lam: float, use_f32r: bool = USE_F32R) -> bass.Bass:
    key = (lam, use_f32r)
    if key not in _PROGRAM_CACHE:
        _PROGRAM_CACHE[key] = build_program(lam, use_f32r)
    return _PROGRAM_CACHE[key]


def _tile_rows(x: np.ndarray) -> np.ndarray:
    """[384, N] row-major -> [128, 3, N] partition-tiled layout."""
    return np.ascontiguousarray(x.reshape(NT, P, -1).transpose(1, 0, 2))


def _untile_rows(x: np.ndarray) -> np.ndarray:
    """[128, 3, N] partition-tiled -> [384, N]."""
    return x.transpose(1, 0, 2).reshape(N, -1)


def make_in_maps(od, adj, dist):
    adj_u8t = _tile_rows(adj.astype(np.uint8))
    dist_t = _tile_rows(dist)
    in_maps = []
    for i in range(NCORES):
        sl = slice(S * i, S * (i + 1))
        ne = np.ones((S, N), np.float32)
        ne[np.arange(S), np.arange(S * i, S * i + S)] = 0.0
        aux = np.stack(
            [
                adj[sl].view(np.float32),
                dist[sl],
                od[sl],
                ne,
            ]
        )
        in_maps.append(
            {
                "adj_u8t": adj_u8t,
                "dist_t": dist_t,
                "aux_s": np.ascontiguousarray(aux),
            }
        )
    return in_maps


def gather(results) -> np.ndarray:
    out = np.zeros((N, N), np.float32)
    for i in range(NCORES):
        out += _untile_rows(results[i]["p3_t"])
        out[S * i : S * i + S] += results[i]["rows"]
    return out


def kernel(od, adj, dist, lambda_param, capacity=None, **_unused) -> np.ndarray:
    od = np.ascontiguousarray(np.asarray(od, dtype=np.float32))
    adj = np.ascontiguousarray(np.asarray(adj, dtype=np.int32))
    dist = np.ascontiguousarray(np.asarray(dist, dtype=np.float32))
    lam = float(np.asarray(lambda_param))
    nc = _get_program(lam)
    res = run_bass_kernel_spmd(nc, make_in_maps(od, adj, dist), list(range(NCORES)))
    return gather(res.results)


# revision 18
# speedup vs baseline: 1.0413x; 1.0413x over previous
"""Trainium2 Bass kernel for the bilevel logit-assignment flow problem.

Reference math (N=384, cutoff-2 paths):
    A = (adj > 0) & ~eye
    E = A * exp(-lam * dist)                       # "edge weight" matrix
    Z = E + offdiag(E @ E)                          # softmax denominator (m cancels)
    W = where(Z > 0, demand / Z, 0),  demand = relu(od) (diag auto-zero via Z)
    flows = W*E + E*(W @ E^T) + E*(E^T @ W)

Sharding: origin axis o split across 8 cores (48 rows each). Each core
holds full E / E^T (N x N is small), computes its row-slice of Z/W and
the three matmuls, and returns:
    rows [48,384] = E_s * (W_s + W_s @ E^T)        # terms 1+2, rows S
    p3   [384,384] = E * (E_s^T @ W_s)             # term 3 partial (sum over its o-slice)
Host gather: flows = sum_i p3_i; flows[S_i] += rows_i.

Input marshaling (host side, layout only): adjacency is repacked to
uint8 (binary matrix, 4x fewer DMA bytes) and adj/dist/p3 use a
partition-tiled [128, 3, 384] DRAM layout so each partition's DMA row
is contiguous.
"""

import numpy as np

import concourse.bass as bass
import concourse.mybir as mybir
import concourse.tile as tile
from concourse import bacc
from concourse.bass_utils import run_bass_kernel_spmd
from concourse.masks import make_identity

N = 384
NCORES = 8
S = N // NCORES  # 48 origins per core
P = 128
NT = N // P  # 3 partition tiles

F32 = mybir.dt.float32
F32R = mybir.dt.float32r
I32 = mybir.dt.int32
U8 = mybir.dt.uint8
Act = mybir.ActivationFunctionType
Alu = mybir.AluOpType

USE_F32R = True  # fp32r: 1 cyc/row matmul (vs 4 for fp32), operands f32r-rounded


def build_program(lam: float, use_f32r: bool = USE_F32R) -> bass.Bass:
    nc = bacc.Bacc(
        "TRN2",
        target_bir_lowering=False,
        debug=False,
        num_devices=NCORES,
        enable_asserts=False,
    )

    def asmm(ap):
        """View an SBUF AP in the dtype fed to the tensor engine."""
        return ap.bitcast(F32R) if use_f32r else ap

    # partition-tiled layouts: [p, t, n] == full[128*t + p, n]
    adj = nc.dram_tensor("adj_u8t", [P, NT, N], U8, kind="ExternalInput")
    dist = nc.dram_tensor("dist_t", [P, NT, N], F32, kind="ExternalInput")
    # per-core slice pack: [adj_s bits, dist_s, od_s, noteye_s] as f32 planes
    aux_s = nc.dram_tensor("aux_s", [4, S, N], F32, kind="ExternalInput")
    p3 = nc.dram_tensor("p3_t", [P, NT, N], F32, kind="ExternalOutput")
    rows = nc.dram_tensor("rows", [S, N], F32, kind="ExternalOutput")

    aux_r = aux_s.rearrange("k s n -> s k n")  # [48, 4, 384]

    with tile.TileContext(nc) as tc:
        with (
            tc.tile_pool(name="persist", bufs=1) as sb,
            tc.tile_pool(name="work", bufs=2) as work,
            tc.tile_pool(name="pst", bufs=2, space="PSUM") as pst,
            tc.tile_pool(name="psacc", bufs=1, space="PSUM") as psacc,
        ):
            ident = sb.tile([P, P], F32)
            make_identity(nc, ident[:])
            ident_mm = sb.tile([P, P], F32)
            nc.vector.tensor_copy(asmm(ident_mm[:]), ident[:])

            # ---- loads (issue split across the two HWDGE engines) ----
            adj_t = work.tile([P, NT, N], U8, tag="adj_t")
            dist_sb = work.tile([P, NT, N], F32, tag="dist_sb")
            aux = sb.tile([S, 4, N], F32)
            nc.sync.dma_start(adj_t[:], adj[:])
            nc.scalar.dma_start(aux[:], aux_r)
            nc.sync.dma_start(dist_sb[:, 0, :], dist[:, 0, :])
            nc.scalar.dma_start(dist_sb[:, 1, :], dist[:, 1, :])
            nc.sync.dma_start(dist_sb[:, 2, :], dist[:, 2, :])

            # ---- per-core row slice E_s (first: only needs aux) ----
            ne_t = aux[:, 3, :]
            Es = sb.tile([S, N], F32)
            adjsf = work.tile([S, N], F32, tag="adjsf")
            nc.vector.tensor_copy(adjsf[:], aux[:, 0, :].bitcast(I32))
            nc.vector.tensor_mul(adjsf[:], adjsf[:], ne_t)  # zero diag (core-dep)
            expds = work.tile([S, N], F32, tag="expds")
            nc.scalar.activation(expds[:], aux[:, 1, :], Act.Exp, scale=-lam)
            nc.vector.tensor_mul(asmm(Es[:]), adjsf[:], expds[:])

            # E_s^T [N, S] as NT chunks of [128, S]
            EsT = sb.tile([P, NT, S], F32)
            for c in range(NT):
                tp2 = pst.tile([P, S], F32, tag="tp2")
                nc.tensor.transpose(
                    asmm(tp2[:]),
                    asmm(Es[:, P * c : P * (c + 1)]),
                    asmm(ident_mm[:S, :S]),
                )
                nc.vector.tensor_copy(asmm(EsT[:, c, :]), tp2[:])

            # ---- full E (pipelined per row-tile), mm(i) folded in ----
            E = sb.tile([P, NT, N], F32)   # E[p, t, :] == E_full[128*t + p, :]
            ET = sb.tile([P, NT, N], F32)  # ET[p, u, :] == E_full[:, 128*u + p].T
            adjf = work.tile([P, NT, N], F32, tag="adjf")
            expd = work.tile([P, NT, N], F32, tag="expd")
            EEs = psacc.tile([S, N], F32, tag="EEs")
            for t in range(NT):
                nc.vector.tensor_copy(adjf[:, t, :], adj_t[:, t, :])  # u8 -> f32
                # zero the global diagonal: iota = 128*t + p - y
                nc.gpsimd.affine_select(
                    out=adjf[:, t, :],
                    in_=adjf[:, t, :],
                    compare_op=Alu.not_equal,
                    fill=0.0,
                    base=P * t,
                    channel_multiplier=1,
                    pattern=[[-1, N]],
                )
                nc.scalar.activation(expd[:, t, :], dist_sb[:, t, :], Act.Exp, scale=-lam)
                nc.vector.tensor_mul(asmm(E[:, t, :]), adjf[:, t, :], expd[:, t, :])
                # (i) EEs = (E @ E)[S, :], accumulated as E row-tiles complete
                nc.tensor.matmul(
                    EEs[:],
                    asmm(EsT[:, t, :]),
                    asmm(E[:, t, :]),
                    start=(t == 0),
                    stop=(t == NT - 1),
                )

            # ET transposes, u-major; pairs share a PSUM tile to halve the copies
            for u in range(NT):
                tp = pst.tile([P, 2, P], F32, tag="tp")
                for t in range(2):
                    nc.tensor.transpose(
                        asmm(tp[:, t, :]),
                        asmm(E[:, t, P * u : P * (u + 1)]),
                        asmm(ident_mm[:]),
                    )
                nc.scalar.copy(asmm(ET[:, u, 0 : 2 * P]), tp[:])
                tp1 = pst.tile([P, 2, P], F32, tag="tp")
                nc.tensor.transpose(
                    asmm(tp1[:, 0, :]),
                    asmm(E[:, 2, P * u : P * (u + 1)]),
                    asmm(ident_mm[:]),
                )
                nc.scalar.copy(asmm(ET[:, u, 2 * P : N]), tp1[:, 0, :])

            # ---- Z, W (whole-tensor chain; 1-input pieces on gpsimd) ----
            dem = work.tile([S, N], F32, tag="dem")
            nc.vector.tensor_relu(dem[:], aux[:, 2, :])
            Zs = sb.tile([S, N], F32)
            mask = work.tile([S, N], F32, tag="mask")
            zinv = work.tile([S, N], F32, tag="zinv")
            W = sb.tile([S, N], F32)
            WsT = sb.tile([P, NT, S], F32)
            T2 = psacc.tile([S, N], F32, tag="T2")
            nc.vector.tensor_add(Zs[:], Es[:], EEs[:])
            nc.vector.tensor_mul(Zs[:], Zs[:], ne_t)  # offdiag()
            nc.vector.tensor_single_scalar(mask[:], Zs[:], 0.0, Alu.is_gt)
            nc.vector.tensor_mul(dem[:], dem[:], mask[:])
            nc.vector.tensor_scalar_max(Zs[:], Zs[:], 1e-30)
            nc.vector.reciprocal(zinv[:], Zs[:])
            nc.vector.tensor_mul(asmm(W[:]), dem[:], zinv[:])
            for c in range(NT):
                cs = slice(P * c, P * (c + 1))
                tp2 = pst.tile([P, S], F32, tag="tp2")
                nc.tensor.transpose(
                    asmm(tp2[:]), asmm(W[:, cs]), asmm(ident_mm[:S, :S])
                )
                nc.scalar.copy(asmm(WsT[:, c, :]), tp2[:])
                # (ii) T2 = W_s @ E^T
                nc.tensor.matmul(
                    T2[:],
                    asmm(WsT[:, c, :]),
                    asmm(ET[:, c, :]),
                    start=(c == 0),
                    stop=(c == NT - 1),
                )

            # ---- rows out ----
            rows_sb = work.tile([S, N], F32, tag="rows_sb")
            nc.vector.tensor_add(rows_sb[:], W[:], T2[:])
            nc.vector.tensor_mul(rows_sb[:], rows_sb[:], Es[:])
            nc.scalar.dma_start(rows[:, :], rows_sb[:])

            # ---- (iii) P3 = E_s^T @ W_s, p3 = E * P3 (early, per-tile out) ----
            for mt in range(NT):
                P3 = pst.tile([P, N], F32, tag="P3")
                nc.tensor.matmul(
                    P3[:],
                    asmm(Es[:, P * mt : P * (mt + 1)]),
                    asmm(W[:]),
                    start=True,
                    stop=True,
                )
                out_t = work.tile([P, N], F32, tag="out_t")
                nc.vector.tensor_mul(out_t[:], E[:, mt, :], P3[:])
                eng = nc.sync if mt % 2 == 0 else nc.scalar
                eng.dma_start(p3[:, mt, :], out_t[:])


    nc.compile()  # bacc register allocation / DCE / lowering
    return nc


_PROGRAM_CACHE: dict = {}


def _get_program(lam: float, use_f32r: bool = USE_F32R) -> bass.Bass:
    key = (lam, use_f32r)
    if key not in _PROGRAM_CACHE:
        _PROGRAM_CACHE[key] = build_program(lam, use_f32r)
    return _PROGRAM_CACHE[key]


def _tile_rows(x: np.ndarray) -> np.ndarray:
    """[384, N] row-major -> [128, 3, N] partition-tiled layout."""
    return np.ascontiguousarray(x.reshape(NT, P, -1).transpose(1, 0, 2))


def _untile_rows(x: np.ndarray) -> np.ndarray:
    """[128, 3, N] partition-tiled -> [384, N]."""
    return x.transpose(1, 0, 2).reshape(N, -1)


def make_in_maps(od, adj, dist):
    adj_u8t = _tile_rows(adj.astype(np.uint8))
    dist_t = _tile_rows(dist)
    in_maps = []
    for i in range(NCORES):
        sl = slice(S * i, S * (i + 1))
        ne = np.ones((S, N), np.float32)
        ne[np.arange(S), np.arange(S * i, S * i + S)] = 0.0
        aux = np.stack(
            [
                adj[sl].view(np.float32),
                dist[sl],
                od[sl],
                ne,
            ]
        )
        in_maps.append(
            {
                "adj_u8t": adj_u8t,
                "dist_t": dist_t,
                "aux_s": np.ascontiguousarray(aux),
            }
        )
    return in_maps


def gather(results) -> np.ndarray:
    out = np.zeros((N, N), np.float32)
    for i in range(NCORES):
        out += _untile_rows(results[i]["p3_t"])
        out[S * i : S * i + S] += results[i]["rows"]
    return out


def kernel(od, adj, dist, lambda_param, capacity=None, **_unused) -> np.ndarray:
    od = np.ascontiguousarray(np.asarray(od, dtype=np.float32))
    adj = np.ascontiguousarray(np.asarray(adj, dtype=np.int32))
    dist = np.ascontiguousarray(np.asarray(dist, dtype=np.float32))
    lam = float(np.asarray(lambda_param))
    nc = _get_program(lam)
    res = run_bass_kernel_spmd(nc, make_in_maps(od, adj, dist), list(range(NCORES)))
    return gather(res.results)
